# revision 1
# baseline (speedup 1.0000x reference)
"""Multi-head attention (B=2, S=2048, H=16, D=64) on 8 Trainium2 NeuronCores.

Head-parallel tensor parallelism: core c owns heads {2c, 2c+1} (a 128-dim
slice of the model dim): column-parallel QKV projections and local causal
attention for its 2 heads, then an AllToAll of bf16 context vectors (one
half-batch at a time, pipelined behind attention) and a full-width Wo
projection for this core's own disjoint 128-token output slices.

Shaped by trace measurements on this part:

* x loads in 8 per-token-tile DMAs and the QKV projection of tile t is
  interleaved with attention on query group t-1, so the PE starts ~20 us
  earlier than with a monolithic x load, and ACT-bound attention stretches
  overlap projection matmuls.
* Attention-times-V keeps V plus a trailing ones column as the 65-column
  stationary operand and streams the exp tile (one matmul per key block,
  N<=512): context comes out already transposed ([dims, tokens]) and the
  softmax denominator lands on PSUM partition 64.
* Scores use tile_position row pairs: each head is a K=64 matmul on its own
  row-group half of the PE array, so the two heads' score matmuls run
  concurrently (no zero-padded K=128 operands).
* exp is one ACT instruction per key block covering both heads.
* Softmax normalization happens on the *receiving* core: the AllToAll
  payload is 130 rows per peer (65 per head: 64 unnormalized ctx dims plus
  the denominator row), so the 16 denominator rows stack on the partition
  axis at the receiver where one 16-lane DVE reciprocal + a DRAM-bounced
  stride-0 broadcast + one fused multiply normalize the gathered ctx.
  (DVE reciprocal is ~8 cycles/element/lane, so sender-side row-wise
  reciprocals were 3.3 us each; gpsimd partition_broadcast and the custom
  reciprocal_approx_fast DVE op both produce wrong results on hardware.)
* Collective triggers block the GpSimd queue until the collective
  completes, so nothing else is ever placed on GpSimd, and each half-batch
  Wo projection is emitted two sections after its AllToAll was issued.
* A tiny warm-up AllToAll is issued during the load phase so the first real
  collective doesn't pay the ~23 us first-call setup on the critical path.
"""

import sys

sys.path.insert(0, "/opt/trn_rl_repo")

import ml_dtypes
import numpy as np

import concourse.bass as bass
import concourse.tile as tile
from concourse import bacc, mybir
from concourse.bass_utils import run_bass_kernel_spmd

N_CORES = 8
B, S, H, D = 2, 2048, 16, 64
E = H * D            # 1024
T = B * S            # 4096 tokens
DPC = 128            # dims (2 heads) per core
NKC = E // 128       # 8 contraction chunks for the projections
NTT = T // 512       # 8 token tiles of 512
SB = S // 128        # 16 key blocks per batch
PH = S // 2 // N_CORES  # 128 tokens per core per half-batch
CR = 130             # a2a chunk rows: 2 x (64 ctx dims + den)

F32 = mybir.dt.float32
BF16 = mybir.dt.bfloat16
AFT = mybir.ActivationFunctionType


def build_program():
    nc = bacc.Bacc("TRN2", target_bir_lowering=False, debug=False,
                   num_devices=N_CORES)

    xT = nc.dram_tensor("xT", [E, T], BF16, kind="ExternalInput").ap()
    wqT = nc.dram_tensor("wqT", [E, DPC], BF16, kind="ExternalInput").ap()
    wkT = nc.dram_tensor("wkT", [E, DPC], BF16, kind="ExternalInput").ap()
    wvT = nc.dram_tensor("wvT", [E, DPC], BF16, kind="ExternalInput").ap()
    woT = nc.dram_tensor("woT", [E, E], BF16, kind="ExternalInput").ap()
    bq = nc.dram_tensor("bq", [DPC, 1], F32, kind="ExternalInput").ap()
    bk = nc.dram_tensor("bk", [DPC, 1], F32, kind="ExternalInput").ap()
    bv = nc.dram_tensor("bv", [DPC, 1], F32, kind="ExternalInput").ap()
    bo = nc.dram_tensor("bo", [E], F32, kind="ExternalInput").ap()
    # single 128x128 lower-triangular (k_local <= q_local) mask
    tri = nc.dram_tensor("tri", [128, 128], BF16, kind="ExternalInput").ap()
    ident = nc.dram_tensor("ident", [128, 128], BF16, kind="ExternalInput").ap()
    out = nc.dram_tensor("out", [T // N_CORES, E], F32, kind="ExternalOutput").ap()

    with tile.TileContext(nc) as tc:
        with (
            tc.tile_pool(name="consts", bufs=1) as consts,
            tc.tile_pool(name="state", bufs=1) as state,
            tc.tile_pool(name="ep", bufs=6) as ep,
            tc.tile_pool(name="op", bufs=4) as op,
            tc.tile_pool(name="ps_s", bufs=2, space="PSUM") as ps_s,
            tc.tile_pool(name="ps_c", bufs=3, space="PSUM") as ps_c,
            tc.tile_pool(name="ps_t", bufs=1, space="PSUM") as ps_t,
            tc.tile_pool(name="dram", bufs=1, space="DRAM") as dram,
        ):
            # ---- warm-up collective: absorbs the first-AllToAll setup cost
            # while the DMA engines are still loading x ----------------------
            wu_s = consts.tile([128, 16], BF16)
            nc.vector.memset(wu_s[:], 0.0)
            wu_in = dram.tile([N_CORES, 16, 16], BF16, tag="wu_in", name="wu_in")
            wu_out = dram.tile([N_CORES, 16, 16], BF16, tag="wu_out",
                               name="wu_out")
            nc.sync.dma_start(out=wu_in[:], in_=wu_s[:])
            nc.gpsimd.collective_compute(
                "AllToAll",
                mybir.AluOpType.bypass,
                replica_groups=[list(range(N_CORES))],
                ins=[wu_in.opt()],
                outs=[wu_out.opt()],
            )

            # ---- constants (one DMA per tensor) ---------------------------
            def chunked(dram_ap, cols):
                # DRAM [E, cols] viewed as [p, kc, cols]: row kc*128+p
                return bass.AP(tensor=dram_ap.tensor, offset=dram_ap.offset,
                               ap=[[cols, 128], [128 * cols, NKC], [1, cols]])

            wq_sb = consts.tile([128, NKC, DPC], BF16)
            wk_sb = consts.tile([128, NKC, DPC], BF16)
            wv_sb = consts.tile([128, NKC, DPC], BF16)
            nc.sync.dma_start(out=wq_sb[:], in_=chunked(wqT, DPC))
            nc.sync.dma_start(out=wk_sb[:], in_=chunked(wkT, DPC))
            nc.sync.dma_start(out=wv_sb[:], in_=chunked(wvT, DPC))
            bq_sb = consts.tile([128, 1], F32)
            bk_sb = consts.tile([128, 1], F32)
            bv_sb = consts.tile([128, 1], F32)
            nc.sync.dma_start(out=bq_sb[:], in_=bq[:])
            nc.sync.dma_start(out=bk_sb[:], in_=bk[:])
            nc.sync.dma_start(out=bv_sb[:], in_=bv[:])
            bo_bc = consts.tile([128, E], F32)
            nc.sync.dma_start(
                out=bo_bc[:],
                in_=bass.AP(tensor=bo.tensor, offset=bo.offset,
                            ap=[[0, 128], [1, E]]),
            )
            tri_sb = consts.tile([128, 128], BF16)
            nc.sync.dma_start(out=tri_sb[:], in_=tri[:])
            id_sb = consts.tile([128, 128], BF16)
            nc.sync.dma_start(out=id_sb[:], in_=ident[:])

            # ---- x, one tile per 512-token group.  DMA issue order
            # interleaves batch-0 and batch-1 tiles (0,4,1,5,...) so batch
            # 1's tiles land before their projections need them instead of
            # draining last behind all of batch 0's bulk. ------------------
            x_t = [None] * NTT
            for tt in (0, 4, 1, 5, 2, 6, 3, 7):
                xt = state.tile([128, NKC, 512], BF16, name=f"x{tt}")
                nc.sync.dma_start(
                    out=xt[:],
                    in_=bass.AP(tensor=xT.tensor, offset=xT.offset + tt * 512,
                                ap=[[T, 128], [128 * T, NKC], [1, 512]]))
                x_t[tt] = xt
            wo_sb = consts.tile([128, NKC, E], BF16)
            nc.sync.dma_start(out=wo_sb[:], in_=chunked(woT, E))

            # ---- persistent activations -----------------------------------
            qT_sb = state.tile([128, T], BF16)   # [2-head dims, tokens]
            kT_sb = state.tile([128, T], BF16)
            vT_sb = state.tile([128, T], BF16)
            # per 128-token block: [64 v-dims, ones] per head -> the AV
            # matmul's 65-column stationary operand; the ones column makes
            # PSUM row 64 the softmax denominator.
            vN_sb = state.tile([128, T // 128, 130], BF16)
            # unnormalized ctx^T + den: rows 0-63 ctx dims, row 64 den
            ctx2_sb = state.tile([65, 2, T], BF16)

            nc.vector.memset(vN_sb[:, :, 64:65], 1.0)
            nc.vector.memset(vN_sb[:, :, 129:130], 1.0)

            # ---- stage builders -------------------------------------------
            def emit_proj(tt):
                ts = slice(tt * 512, (tt + 1) * 512)
                ps_qk = ps_s.tile([128, 2, 512], F32, tag="s", name="ps_qk")
                for kc in range(NKC):
                    nc.tensor.matmul(ps_qk[:, 0, :], wq_sb[:, kc, :],
                                     x_t[tt][:, kc, :],
                                     start=(kc == 0), stop=(kc == NKC - 1),
                                     skip_group_check=True)
                for kc in range(NKC):
                    nc.tensor.matmul(ps_qk[:, 1, :], wk_sb[:, kc, :],
                                     x_t[tt][:, kc, :],
                                     start=(kc == 0), stop=(kc == NKC - 1),
                                     skip_group_check=True)
                ps_v = ps_s.tile([128, 2, 512], F32, tag="s", name="ps_v")
                for kc in range(NKC):
                    nc.tensor.matmul(ps_v[:, 0, :], wv_sb[:, kc, :],
                                     x_t[tt][:, kc, :],
                                     start=(kc == 0), stop=(kc == NKC - 1),
                                     skip_group_check=True)
                nc.vector.tensor_scalar_add(qT_sb[:, ts], ps_qk[:, 0, :],
                                            bq_sb[:])
                nc.vector.tensor_scalar_add(kT_sb[:, ts], ps_qk[:, 1, :],
                                            bk_sb[:])
                nc.vector.tensor_scalar_add(vT_sb[:, ts], ps_v[:, 0, :],
                                            bv_sb[:])
                for tb in range(tt * 4, tt * 4 + 4):
                    tp_ps = ps_t.tile([128, 128], BF16, tag="tp", name="tp_ps")
                    nc.tensor.transpose(
                        tp_ps[:], vT_sb[:, tb * 128:(tb + 1) * 128], id_sb[:])
                    nc.vector.tensor_copy(vN_sb[:, tb, 0:64], tp_ps[:, 0:64])
                    nc.vector.tensor_copy(vN_sb[:, tb, 65:129],
                                          tp_ps[:, 64:128])

            def emit_attn(b, qt):
                t0 = b * S
                q0 = t0 + qt * 512
                nkb = 4 * qt + 4

                def emit_scores(kb):
                    c0 = max(kb - 4 * qt, 0) * 128
                    s = ps_s.tile([128, 2, 512], F32, tag="s", name="s_ps")
                    for h in range(2):
                        d0 = h * 64
                        nc.tensor.matmul(
                            s[:, h, c0:512],
                            kT_sb[d0:d0 + 64,
                                  t0 + kb * 128:t0 + (kb + 1) * 128],
                            qT_sb[d0:d0 + 64, q0 + c0:q0 + 512],
                            start=True, stop=True, skip_group_check=True)
                    return s

                s_tiles = {0: emit_scores(0)}
                cn = [ps_c.tile([128, 512], F32, tag="cn", name=f"cn{h}")
                      for h in range(2)]
                for kb in range(nkb):
                    m = kb - 4 * qt
                    c0 = max(m, 0) * 128
                    if kb + 1 < nkb:
                        s_tiles[kb + 1] = emit_scores(kb + 1)
                    s = s_tiles.pop(kb)
                    e = ep.tile([128, 2, 512], BF16, tag="e", name="e_sb")
                    nc.scalar.activation(e[:, :, c0:512], s[:, :, c0:512],
                                         AFT.Exp, scale=0.125)
                    if m >= 0:  # triangular block on the diagonal
                        for h in range(2):
                            nc.vector.tensor_mul(e[:, h, c0:c0 + 128],
                                                 e[:, h, c0:c0 + 128],
                                                 tri_sb[:])
                    for h in range(2):
                        nc.tensor.matmul(
                            cn[h][0:65, c0:512],
                            vN_sb[:, b * SB + kb, 65 * h:65 * h + 65],
                            e[:, h, c0:512],
                            start=(kb == 0), stop=(kb == nkb - 1),
                            skip_group_check=True)

                # stage unnormalized ctx + den rows for the AllToAll
                for h in range(2):
                    nc.vector.tensor_copy(ctx2_sb[:, h, q0:q0 + 512],
                                          cn[h][0:65, :])

            def emit_half_a2a(b, hf):
                base = b * S + hf * (S // 2)
                ctxd = dram.tile([N_CORES, CR, PH], BF16, tag="ctxd",
                                 name="ctxd", bufs=4)
                for j in range(N_CORES):
                    dst = ctxd[j]
                    nc.sync.dma_start(
                        out=bass.AP(tensor=dst.tensor, offset=dst.offset,
                                    ap=[[PH, 65], [65 * PH, 2], [1, PH]]),
                        in_=ctx2_sb[:, :, base + j * PH:base + (j + 1) * PH])
                recv = dram.tile([N_CORES, CR, PH], BF16, tag="recv",
                                 name="recv", bufs=4)
                nc.gpsimd.collective_compute(
                    "AllToAll",
                    mybir.AluOpType.bypass,
                    replica_groups=[list(range(N_CORES))],
                    ins=[ctxd.opt()],
                    outs=[recv.opt()],
                )
                return recv

            def emit_half_recv(b, hf, recv):
                # gather + normalize the received ctx; no PE work, so the PE
                # queue never parks on this chain
                r0 = recv[0]
                cg_sb = op.tile([128, NKC, PH], BF16, tag="cg_sb", name="cg_sb",
                                bufs=2)
                for h in range(2):
                    nc.sync.dma_start(
                        out=cg_sb[h * 64:(h + 1) * 64, :, :],
                        in_=bass.AP(tensor=r0.tensor,
                                    offset=r0.offset + h * 65 * PH,
                                    ap=[[PH, 64], [CR * PH, N_CORES],
                                        [1, PH]]))
                # 16 denominator rows stacked on partitions: p = 2*j + h
                den16 = op.tile([16, PH], BF16, tag="den16", name="den16",
                                bufs=2)
                nc.sync.dma_start(
                    out=den16[:],
                    in_=bass.AP(tensor=r0.tensor, offset=r0.offset + 64 * PH,
                                ap=[[CR * PH, N_CORES], [65 * PH, 2],
                                    [1, PH]]))
                r16 = op.tile([16, PH], F32, tag="r16", name="r16", bufs=2)
                nc.vector.reciprocal(r16[:], den16[:])
                rd = dram.tile([16, PH], F32, tag="rd", name="rd", bufs=4)
                nc.sync.dma_start(out=rd[:], in_=r16[:])
                rmap = op.tile([128, NKC, PH], F32, tag="rmap", name="rmap",
                               bufs=2)
                rd0 = rd[0]
                for h in range(2):
                    nc.sync.dma_start(
                        out=rmap[h * 64:(h + 1) * 64, :, :],
                        in_=bass.AP(tensor=rd0.tensor,
                                    offset=rd0.offset + h * PH,
                                    ap=[[0, 64], [2 * PH, N_CORES], [1, PH]]))
                nc.vector.tensor_mul(cg_sb[:], cg_sb[:], rmap[:])
                return b, hf, cg_sb

            def emit_half_wo(b, hf, cg_sb):
                o_sb = op.tile([PH, E], F32, tag="o_sb", name="o_sb")
                for et in range(2):
                    ps = ps_s.tile([128, 2, 512], F32, tag="s", name="c_ps")
                    for kc in range(NKC):
                        nc.tensor.matmul(
                            ps[0:PH, 0, :],
                            cg_sb[:, kc, :],
                            wo_sb[:, kc, et * 512:(et + 1) * 512],
                            start=(kc == 0), stop=(kc == NKC - 1),
                            skip_group_check=True)
                    nc.vector.tensor_add(
                        o_sb[:, et * 512:(et + 1) * 512], ps[0:PH, 0, :],
                        bo_bc[0:PH, et * 512:(et + 1) * 512])
                r0w = (b * 2 + hf) * PH
                nc.sync.dma_start(out=out[r0w:r0w + PH, :], in_=o_sb[:])

            # ---- interleaved schedule -------------------------------------
            # Per half-batch: A2A issued at its boundary; gather+normalize one
            # boundary later (collective long done); Wo matmuls one boundary
            # after that (inputs ready the moment the PE reaches them).
            a2aq = []  # (b, hf, recv): A2A issued, recv-chain not emitted
            woq = []   # (b, hf, cg_sb): normalized, Wo matmuls not emitted

            for b in range(B):
                for qt in range(4):
                    emit_proj(b * 4 + qt)
                    emit_attn(b, qt)
                    if qt in (1, 3):
                        if woq:
                            emit_half_wo(*woq.pop(0))
                        if a2aq:
                            woq.append(emit_half_recv(*a2aq.pop(0)))
                        a2aq.append((b, qt // 2,
                                     emit_half_a2a(b, qt // 2)))
            while woq or a2aq:
                if woq:
                    emit_half_wo(*woq.pop(0))
                if a2aq:
                    woq.append(emit_half_recv(*a2aq.pop(0)))

    nc.compile()
    return nc


_NC = None


def _get_program():
    global _NC
    if _NC is None:
        _NC = build_program()
    return _NC


def _bf(a):
    return np.ascontiguousarray(a).astype(ml_dtypes.bfloat16)


def kernel(x, Wq, bq, Wk, bk, Wv, bv, Wo, bo, _trace=False, _trace_kwargs=None):
    x = np.asarray(x, np.float32)
    Wq, Wk, Wv, Wo = (np.asarray(w, np.float32) for w in (Wq, Wk, Wv, Wo))
    bq, bk, bv, bo = (np.asarray(v, np.float32) for v in (bq, bk, bv, bo))

    xT = _bf(x.reshape(T, E).T)
    i = np.arange(128)
    tri = _bf((i[:, None] <= i[None, :]).astype(np.float32))
    ident = _bf(np.eye(128, dtype=np.float32))

    in_maps = []
    for c in range(N_CORES):
        sl = slice(c * DPC, (c + 1) * DPC)
        in_maps.append({
            "xT": xT,
            "wqT": _bf(Wq[sl, :].T),
            "wkT": _bf(Wk[sl, :].T),
            "wvT": _bf(Wv[sl, :].T),
            "woT": _bf(Wo.T),
            "bq": bq[sl].reshape(DPC, 1).copy(),
            "bk": bk[sl].reshape(DPC, 1).copy(),
            "bv": bv[sl].reshape(DPC, 1).copy(),
            "bo": bo,
            "tri": tri,
            "ident": ident,
        })

    nc = _get_program()
    res = run_bass_kernel_spmd(nc, in_maps, list(range(N_CORES)),
                               trace=_trace, **(_trace_kwargs or {}))
    # out[c] rows are [batch, half, 128]: row (b, hf, r) holds global
    # token b*2048 + hf*1024 + c*128 + r.
    stacked = np.stack([res.results[i]["out"].reshape(B, 2, 128, E)
                        for i in range(N_CORES)], axis=2)
    full = stacked.reshape(T, E)
    if _trace:
        return full.reshape(B, S, E), res
    return full.reshape(B, S, E)



# revision 3
# speedup vs baseline: 1.0185x; 1.0185x over previous
"""Multi-head attention (B=2, S=2048, H=16, D=64) on 8 Trainium2 NeuronCores.

Head-parallel tensor parallelism: core c owns heads {2c, 2c+1} (a 128-dim
slice of the model dim): column-parallel QKV projections and local causal
attention for its 2 heads, then an AllToAll of bf16 context vectors (one
512-token query group at a time) and a full-width Wo projection for this
core's own disjoint 64-token output slices.

Schedule (v2), shaped by trace measurements:

* Startup: wq/bq and the first half of x's token-tile 0 are the first DMAs
  issued (the x0 tile is split in two so the first projection matmuls start
  after ~0.75 MB of transfers instead of ~3 MB).  Everything else loads
  behind them in deadline order (tri before the first diagonal block,
  wo/bo last).
* Query groups run in order {0, 1, 3, 2} per batch and each group's
  unnormalized ctx + softmax denominators are AllToAll'd on their own
  (8 collectives of 133 KB instead of 4 of 266 KB), so the tail of the
  kernel only waits for one small collective.  The receive-side
  gather/normalize + Wo projection stays at half-batch granularity
  (per-qt Wo would halve the PE array's M occupancy).
* PSUM: proj pool 2 banks (q/k/v/transpose rotate through [128,512] slots),
  scores 2x2 banks, ctx accumulators 2 banks = exactly 8.  The projection
  for tile t+1 and the next half-batch's Wo matmuls are emitted after the
  attention section that hides them; the Tile scheduler slots them into
  the PE stalls where attention waits on the ACT exp stream (exp on ACT is
  ~1.15 us per 128-key block vs ~0.65 us of PE work, so without filler the
  PE idles ~40% during attention and the HAM clock gate re-throttles).
* batch-1 half-0's Wo matmuls are emitted after the LAST a2a is issued so
  the PE stays busy (and warm) through the final collective.
* Softmax normalization happens on the receiving core (the a2a payload is
  65 rows per head: 64 unnormalized ctx dims + the denominator row from a
  trailing ones-column in the AV stationary); 16 denominator rows stack on
  the partition axis at the receiver where one 16-lane DVE reciprocal + a
  DRAM-bounced stride-0 broadcast + one fused multiply normalize the
  gathered ctx.
* Attention-times-V keeps V plus a trailing ones column as the 65-column
  stationary operand and streams the exp tile; scores use tile_position
  row pairs so the two heads' score matmuls run concurrently; exp is one
  ACT instruction per key block covering both heads; the diagonal tri-mask
  is one DVE multiply per block via a stride-0 broadcast AP over heads.
* A tiny warm-up AllToAll is issued during the load phase so the first real
  collective doesn't pay the ~23 us first-call setup on the critical path.
"""

import sys

sys.path.insert(0, "/opt/trn_rl_repo")

import ml_dtypes
import numpy as np

import concourse.bass as bass
import concourse.tile as tile
from concourse import bacc, mybir
from concourse.bass_utils import run_bass_kernel_spmd

N_CORES = 8
B, S, H, D = 2, 2048, 16, 64
E = H * D            # 1024
T = B * S            # 4096 tokens
DPC = 128            # dims (2 heads) per core
NKC = E // 128       # 8 contraction chunks for the projections
SB = S // 128        # 16 key blocks per batch
PHQ = 512 // N_CORES  # 64 tokens per core per query group
PH = 2 * PHQ         # 128 tokens per core per half-batch
CR = 130             # a2a chunk rows: 2 x (64 ctx dims + den)

F32 = mybir.dt.float32
BF16 = mybir.dt.bfloat16
AFT = mybir.ActivationFunctionType

QT_ORDER = (0, 1, 3, 2)  # hf0 = {0,1} finishes early; q2 (12 blocks) last


def build_program():
    nc = bacc.Bacc("TRN2", target_bir_lowering=False, debug=False,
                   num_devices=N_CORES)

    xT = nc.dram_tensor("xT", [E, T], BF16, kind="ExternalInput").ap()
    wqT = nc.dram_tensor("wqT", [E, DPC], BF16, kind="ExternalInput").ap()
    wkT = nc.dram_tensor("wkT", [E, DPC], BF16, kind="ExternalInput").ap()
    wvT = nc.dram_tensor("wvT", [E, DPC], BF16, kind="ExternalInput").ap()
    woT = nc.dram_tensor("woT", [E, E], BF16, kind="ExternalInput").ap()
    bq = nc.dram_tensor("bq", [DPC, 1], F32, kind="ExternalInput").ap()
    bk = nc.dram_tensor("bk", [DPC, 1], F32, kind="ExternalInput").ap()
    bv = nc.dram_tensor("bv", [DPC, 1], F32, kind="ExternalInput").ap()
    bo = nc.dram_tensor("bo", [E], F32, kind="ExternalInput").ap()
    # single 128x128 lower-triangular (k_local <= q_local) mask
    tri = nc.dram_tensor("tri", [128, 128], BF16, kind="ExternalInput").ap()
    ident = nc.dram_tensor("ident", [128, 128], BF16, kind="ExternalInput").ap()
    out = nc.dram_tensor("out", [T // N_CORES, E], F32, kind="ExternalOutput").ap()

    with tile.TileContext(nc) as tc:
        with (
            tc.tile_pool(name="consts", bufs=1) as consts,
            tc.tile_pool(name="state", bufs=1) as state,
            tc.tile_pool(name="ep", bufs=6) as ep,
            tc.tile_pool(name="op", bufs=4) as op,
            tc.tile_pool(name="ps_p", bufs=2, space="PSUM") as ps_p,
            tc.tile_pool(name="ps_s", bufs=2, space="PSUM") as ps_s,
            tc.tile_pool(name="ps_c", bufs=2, space="PSUM") as ps_c,
            tc.tile_pool(name="dram", bufs=1, space="DRAM") as dram,
        ):
            # ---- warm-up collective: absorbs the first-AllToAll setup cost
            # while the DMA engines are still loading x ----------------------
            wu_s = consts.tile([128, 16], BF16)
            nc.vector.memset(wu_s[:], 0.0)
            wu_in = dram.tile([N_CORES, 16, 16], BF16, tag="wu_in", name="wu_in")
            wu_out = dram.tile([N_CORES, 16, 16], BF16, tag="wu_out",
                               name="wu_out")
            nc.sync.dma_start(out=wu_in[:], in_=wu_s[:])
            nc.gpsimd.collective_compute(
                "AllToAll",
                mybir.AluOpType.bypass,
                replica_groups=[list(range(N_CORES))],
                ins=[wu_in.opt()],
                outs=[wu_out.opt()],
            )

            def chunked(dram_ap, cols, kc0, kcn):
                # DRAM [E, cols] viewed as [p, kc, cols]: row kc*128+p
                return bass.AP(tensor=dram_ap.tensor,
                               offset=dram_ap.offset + kc0 * 128 * cols,
                               ap=[[cols, 128], [128 * cols, kcn], [1, cols]])

            # ---- loads in deadline order ---------------------------------
            wq_sb = consts.tile([128, NKC, DPC], BF16)
            bq_sb = consts.tile([128, 1], F32)
            nc.sync.dma_start(out=wq_sb[:], in_=chunked(wqT, DPC, 0, NKC))
            nc.sync.dma_start(out=bq_sb[:], in_=bq[:])
            # x tile 0 in two halves so the first matmul starts early
            x_t = [None] * NKC
            x0a = state.tile([128, NKC // 2, 512], BF16, name="x0a")
            x0b = state.tile([128, NKC // 2, 512], BF16, name="x0b")

            def x_ap(tt, kc0, kcn):
                return bass.AP(tensor=xT.tensor,
                               offset=xT.offset + tt * 512 + kc0 * 128 * T,
                               ap=[[T, 128], [128 * T, kcn], [1, 512]])

            nc.sync.dma_start(out=x0a[:], in_=x_ap(0, 0, 4))
            wk_sb = consts.tile([128, NKC, DPC], BF16)
            bk_sb = consts.tile([128, 1], F32)
            wv_sb = consts.tile([128, NKC, DPC], BF16)
            bv_sb = consts.tile([128, 1], F32)
            nc.sync.dma_start(out=wk_sb[:], in_=chunked(wkT, DPC, 0, NKC))
            nc.sync.dma_start(out=bk_sb[:], in_=bk[:])
            nc.sync.dma_start(out=x0b[:], in_=x_ap(0, 4, 4))
            nc.sync.dma_start(out=wv_sb[:], in_=chunked(wvT, DPC, 0, NKC))
            nc.sync.dma_start(out=bv_sb[:], in_=bv[:])
            tri_sb = consts.tile([128, 128], BF16)
            nc.sync.dma_start(out=tri_sb[:], in_=tri[:])
            id_sb = consts.tile([128, 128], BF16)
            nc.sync.dma_start(out=id_sb[:], in_=ident[:])
            for tt in range(1, NKC):
                xt = state.tile([128, NKC, 512], BF16, name=f"x{tt}")
                nc.sync.dma_start(out=xt[:], in_=x_ap(tt, 0, NKC))
                x_t[tt] = xt
            wo_sb = consts.tile([128, NKC, E], BF16)
            nc.sync.dma_start(out=wo_sb[:], in_=chunked(woT, E, 0, NKC))
            bo_bc = consts.tile([128, E], F32)
            nc.sync.dma_start(
                out=bo_bc[:],
                in_=bass.AP(tensor=bo.tensor, offset=bo.offset,
                            ap=[[0, 128], [1, E]]),
            )

            # ---- persistent activations -----------------------------------
            qT_sb = state.tile([128, T], BF16)   # [2-head dims, tokens]
            kT_sb = state.tile([128, T], BF16)
            vT_sb = state.tile([128, T], BF16)
            # per 128-token block: [64 v-dims, ones] per head -> the AV
            # matmul's 65-column stationary operand; the ones column makes
            # PSUM row 64 the softmax denominator.
            vN_sb = state.tile([128, T // 128, 130], BF16)
            # unnormalized ctx^T + den: rows 0-63 ctx dims, row 64 den
            ctx2_sb = state.tile([65, 2, T], BF16)

            nc.vector.memset(vN_sb[:, :, 64:65], 1.0)
            nc.vector.memset(vN_sb[:, :, 129:130], 1.0)

            # ---- stage builders -------------------------------------------
            def emit_proj(tt):
                ts = slice(tt * 512, (tt + 1) * 512)
                xa = (x0a, x0b) if tt == 0 else (x_t[tt],)
                nch = NKC // len(xa)

                ps_q = ps_p.tile([128, 512], F32, tag="p", name="ps_q")
                for kc in range(NKC):
                    xt = xa[kc // nch]
                    nc.tensor.matmul(ps_q[:], wq_sb[:, kc, :],
                                     xt[:, kc % nch, :],
                                     start=(kc == 0), stop=(kc == NKC - 1),
                                     skip_group_check=True)
                nc.vector.tensor_scalar_add(qT_sb[:, ts], ps_q[:], bq_sb[:])
                ps_k = ps_p.tile([128, 512], F32, tag="p", name="ps_k")
                for kc in range(NKC):
                    xt = xa[kc // nch]
                    nc.tensor.matmul(ps_k[:], wk_sb[:, kc, :],
                                     xt[:, kc % nch, :],
                                     start=(kc == 0), stop=(kc == NKC - 1),
                                     skip_group_check=True)
                nc.vector.tensor_scalar_add(kT_sb[:, ts], ps_k[:], bk_sb[:])
                ps_v = ps_p.tile([128, 512], F32, tag="p", name="ps_v")
                for kc in range(NKC):
                    xt = xa[kc // nch]
                    nc.tensor.matmul(ps_v[:], wv_sb[:, kc, :],
                                     xt[:, kc % nch, :],
                                     start=(kc == 0), stop=(kc == NKC - 1),
                                     skip_group_check=True)
                nc.vector.tensor_scalar_add(vT_sb[:, ts], ps_v[:], bv_sb[:])
                tp_ps = ps_p.tile([128, 4, 128], BF16, tag="p", name="tp_ps")
                for ti, tb in enumerate(range(tt * 4, tt * 4 + 4)):
                    nc.tensor.transpose(
                        tp_ps[:, ti, :], vT_sb[:, tb * 128:(tb + 1) * 128],
                        id_sb[:])
                    nc.vector.tensor_copy(vN_sb[:, tb, 0:64],
                                          tp_ps[:, ti, 0:64])
                    nc.vector.tensor_copy(vN_sb[:, tb, 65:129],
                                          tp_ps[:, ti, 64:128])

            def emit_attn(b, qt):
                t0 = b * S
                q0 = t0 + qt * 512
                nkb = 4 * qt + 4

                def emit_scores(kb):
                    c0 = max(kb - 4 * qt, 0) * 128
                    s = ps_s.tile([128, 2, 512], F32, tag="s", name="s_ps")
                    for h in range(2):
                        d0 = h * 64
                        nc.tensor.matmul(
                            s[:, h, c0:512],
                            kT_sb[d0:d0 + 64,
                                  t0 + kb * 128:t0 + (kb + 1) * 128],
                            qT_sb[d0:d0 + 64, q0 + c0:q0 + 512],
                            start=True, stop=True, skip_group_check=True)
                    return s

                s_tiles = {0: emit_scores(0)}
                cn = [ps_c.tile([128, 512], F32, tag="cn", name=f"cn{h}")
                      for h in range(2)]
                for kb in range(nkb):
                    m = kb - 4 * qt
                    c0 = max(m, 0) * 128
                    if kb + 1 < nkb:
                        s_tiles[kb + 1] = emit_scores(kb + 1)
                    s = s_tiles.pop(kb)
                    e = ep.tile([128, 2, 512], BF16, tag="e", name="e_sb")
                    nc.scalar.activation(e[:, :, c0:512], s[:, :, c0:512],
                                         AFT.Exp, scale=0.125)
                    if m >= 0:  # triangular block on the diagonal
                        nc.vector.tensor_mul(
                            e[:, :, c0:c0 + 128], e[:, :, c0:c0 + 128],
                            tri_sb[:].unsqueeze(1).broadcast_to((128, 2, 128)))
                    for h in range(2):
                        nc.tensor.matmul(
                            cn[h][0:65, c0:512],
                            vN_sb[:, b * SB + kb, 65 * h:65 * h + 65],
                            e[:, h, c0:512],
                            start=(kb == 0), stop=(kb == nkb - 1),
                            skip_group_check=True)

                # stage unnormalized ctx + den rows for the AllToAll
                for h in range(2):
                    nc.vector.tensor_copy(ctx2_sb[:, h, q0:q0 + 512],
                                          cn[h][0:65, :])

            def emit_qt_a2a(b, qt):
                base = b * S + qt * 512
                ctxd = dram.tile([N_CORES, CR, PHQ], BF16, tag="ctxd",
                                 name="ctxd", bufs=4)
                for h in range(2):
                    nc.sync.dma_start(
                        out=bass.AP(tensor=ctxd.tensor,
                                    offset=ctxd[0].offset + h * 65 * PHQ,
                                    ap=[[PHQ, 65], [CR * PHQ, N_CORES],
                                        [1, PHQ]]),
                        in_=ctx2_sb[:, h, base:base + 512].rearrange(
                            "p (j t) -> p j t", j=N_CORES))
                recv = dram.tile([N_CORES, CR, PHQ], BF16, tag="recv",
                                 name="recv", bufs=4)
                nc.gpsimd.collective_compute(
                    "AllToAll",
                    mybir.AluOpType.bypass,
                    replica_groups=[list(range(N_CORES))],
                    ins=[ctxd.opt()],
                    outs=[recv.opt()],
                )
                return recv

            def emit_half_recv(b, hf, recvs):
                # gather + normalize the received ctx for one half-batch
                # (two query groups); no PE work in this chain.
                cg_sb = op.tile([128, NKC, PH], BF16, tag="cg_sb", name="cg_sb",
                                bufs=2)
                den16 = op.tile([16, PH], BF16, tag="den16", name="den16",
                                bufs=2)
                for ql, recv in enumerate(recvs):
                    r0 = recv[0]
                    for h in range(2):
                        nc.sync.dma_start(
                            out=cg_sb[h * 64:(h + 1) * 64, :,
                                      ql * PHQ:(ql + 1) * PHQ],
                            in_=bass.AP(tensor=r0.tensor,
                                        offset=r0.offset + h * 65 * PHQ,
                                        ap=[[PHQ, 64], [CR * PHQ, N_CORES],
                                            [1, PHQ]]))
                    nc.sync.dma_start(
                        out=den16[:, ql * PHQ:(ql + 1) * PHQ],
                        in_=bass.AP(tensor=r0.tensor,
                                    offset=r0.offset + 64 * PHQ,
                                    ap=[[CR * PHQ, N_CORES], [65 * PHQ, 2],
                                        [1, PHQ]]))
                r16 = op.tile([16, PH], F32, tag="r16", name="r16", bufs=2)
                nc.vector.reciprocal(r16[:], den16[:])
                rd = dram.tile([16, PH], F32, tag="rd", name="rd", bufs=4)
                nc.sync.dma_start(out=rd[:], in_=r16[:])
                rmap = op.tile([128, NKC, PH], F32, tag="rmap", name="rmap",
                               bufs=2)
                rd0 = rd[0]
                for h in range(2):
                    nc.sync.dma_start(
                        out=rmap[h * 64:(h + 1) * 64, :, :],
                        in_=bass.AP(tensor=rd0.tensor,
                                    offset=rd0.offset + h * PH,
                                    ap=[[0, 64], [2 * PH, N_CORES], [1, PH]]))
                nc.vector.tensor_mul(cg_sb[:], cg_sb[:], rmap[:])
                return b, hf, cg_sb

            def emit_half_wo(b, hf, cg_sb):
                o_sb = op.tile([PH, E], F32, tag="o_sb", name="o_sb")
                for et in range(2):
                    ps = ps_s.tile([128, 2, 512], F32, tag="s", name="c_ps")
                    for kc in range(NKC):
                        nc.tensor.matmul(
                            ps[0:PH, 0, :],
                            cg_sb[:, kc, :],
                            wo_sb[:, kc, et * 512:(et + 1) * 512],
                            start=(kc == 0), stop=(kc == NKC - 1),
                            skip_group_check=True)
                    nc.vector.tensor_add(
                        o_sb[:, et * 512:(et + 1) * 512], ps[0:PH, 0, :],
                        bo_bc[0:PH, et * 512:(et + 1) * 512])
                r0w = (b * 2 + hf) * PH
                nc.sync.dma_start(out=out[r0w:r0w + PH, :], in_=o_sb[:])

            # ---- schedule -------------------------------------------------
            # qt order {0,1,3,2}: per batch, the a2a for each query group is
            # issued right after its attention; proj for the next needed
            # tile(s) is emitted after each attention section so the
            # scheduler can fill attention's ACT-bound PE gaps; recv+Wo for
            # each half-batch is emitted 1-2 sections after both its a2as.
            rq = {}   # (b, qt) -> recv tile

            emit_proj(0)
            # -------- batch 0
            emit_attn(0, 0)
            rq[(0, 0)] = emit_qt_a2a(0, 0)
            emit_proj(1)
            emit_attn(0, 1)
            rq[(0, 1)] = emit_qt_a2a(0, 1)
            emit_proj(2)
            emit_proj(3)
            emit_attn(0, 3)
            rq[(0, 3)] = emit_qt_a2a(0, 3)
            emit_proj(4)
            emit_attn(0, 2)
            rq[(0, 2)] = emit_qt_a2a(0, 2)
            emit_proj(5)
            # -------- batch 1
            emit_attn(1, 0)
            rq[(1, 0)] = emit_qt_a2a(1, 0)
            emit_proj(6)
            args00 = emit_half_recv(0, 0, (rq[(0, 0)], rq[(0, 1)]))
            emit_attn(1, 1)
            rq[(1, 1)] = emit_qt_a2a(1, 1)
            emit_proj(7)
            emit_half_wo(*args00)
            args01 = emit_half_recv(0, 1, (rq[(0, 2)], rq[(0, 3)]))
            emit_attn(1, 3)
            rq[(1, 3)] = emit_qt_a2a(1, 3)
            emit_half_wo(*args01)
            emit_attn(1, 2)
            rq[(1, 2)] = emit_qt_a2a(1, 2)
            # batch-1 half 0: recv chain prefetched during the last
            # attention; Wo emitted after the last a2a so the PE works
            # through the collective.
            args10 = emit_half_recv(1, 0, (rq[(1, 0)], rq[(1, 1)]))
            emit_half_wo(*args10)
            args11 = emit_half_recv(1, 1, (rq[(1, 2)], rq[(1, 3)]))
            emit_half_wo(*args11)

    nc.compile()
    return nc


_NC = None


def _get_program():
    global _NC
    if _NC is None:
        _NC = build_program()
    return _NC


def _bf(a):
    return np.ascontiguousarray(a).astype(ml_dtypes.bfloat16)


def kernel(x, Wq, bq, Wk, bk, Wv, bv, Wo, bo, _trace=False, _trace_kwargs=None):
    x = np.asarray(x, np.float32)
    Wq, Wk, Wv, Wo = (np.asarray(w, np.float32) for w in (Wq, Wk, Wv, Wo))
    bq, bk, bv, bo = (np.asarray(v, np.float32) for v in (bq, bk, bv, bo))

    xT = _bf(x.reshape(T, E).T)
    i = np.arange(128)
    tri = _bf((i[:, None] <= i[None, :]).astype(np.float32))
    ident = _bf(np.eye(128, dtype=np.float32))

    in_maps = []
    for c in range(N_CORES):
        sl = slice(c * DPC, (c + 1) * DPC)
        in_maps.append({
            "xT": xT,
            "wqT": _bf(Wq[sl, :].T),
            "wkT": _bf(Wk[sl, :].T),
            "wvT": _bf(Wv[sl, :].T),
            "woT": _bf(Wo.T),
            "bq": bq[sl].reshape(DPC, 1).copy(),
            "bk": bk[sl].reshape(DPC, 1).copy(),
            "bv": bv[sl].reshape(DPC, 1).copy(),
            "bo": bo,
            "tri": tri,
            "ident": ident,
        })

    nc = _get_program()
    res = run_bass_kernel_spmd(nc, in_maps, list(range(N_CORES)),
                               trace=_trace, **(_trace_kwargs or {}))
    # out[c] rows are [batch, qt, 64]: row (b, qt, r) holds global token
    # b*2048 + qt*512 + c*64 + r.
    stacked = np.stack([res.results[i]["out"].reshape(B, 4, PHQ, E)
                        for i in range(N_CORES)], axis=2)
    full = stacked.reshape(T, E)
    if _trace:
        return full.reshape(B, S, E), res
    return full.reshape(B, S, E)


# revision 7
# speedup vs baseline: 1.0598x; 1.0406x over previous
"""Multi-head attention (B=2, S=2048, H=16, D=64) on 8 Trainium2 NeuronCores.

Head-parallel tensor parallelism: core c owns heads {2c, 2c+1} (a 128-dim
slice of the model dim): column-parallel QKV projections and local causal
attention for its 2 heads, then an AllToAll of bf16 context vectors (one
512-token query group at a time) and a full-width Wo projection for this
core's own disjoint 64-token output slices.

Schedule (v2), shaped by trace measurements:

* Startup: wq/bq and the first half of x's token-tile 0 are the first DMAs
  issued (the x0 tile is split in two so the first projection matmuls start
  after ~0.75 MB of transfers instead of ~3 MB).  Everything else loads
  behind them in deadline order (tri before the first diagonal block,
  wo/bo last).
* Query groups run in order {0, 1, 3, 2} per batch; each half-batch's
  unnormalized ctx + softmax denominators are AllToAll'd right when its
  second query group finishes.  4 collectives, spaced ~40 us: per-qt
  granularity (8 collectives) was measured to DEGRADE - back-to-back
  AllToAlls on this part grow from ~6 us to ~22 us each, and the first
  collective completes no earlier than ~100 us (first-call barrier), so
  every collective-dependent Wo block is placed with >=20 us of slack and
  the last two independent Wo blocks are held back as PE filler for the
  final collective.
* PSUM: proj pool 2 banks (q/k/v/transpose rotate through [128,512] slots),
  scores 2x2 banks, ctx accumulators 2 banks = exactly 8.  The projection
  for tile t+1 and the next half-batch's Wo matmuls are emitted after the
  attention section that hides them; the Tile scheduler slots them into
  the PE stalls where attention waits on the ACT exp stream (exp on ACT is
  ~1.15 us per 128-key block vs ~0.65 us of PE work, so without filler the
  PE idles ~40% during attention and the HAM clock gate re-throttles).
* batch-1 half-0's Wo matmuls are emitted after the LAST a2a is issued so
  the PE stays busy (and warm) through the final collective.
* Softmax normalization happens on the receiving core (the a2a payload is
  65 rows per head: 64 unnormalized ctx dims + the denominator row from a
  trailing ones-column in the AV stationary); 16 denominator rows stack on
  the partition axis at the receiver where one 16-lane DVE reciprocal + a
  DRAM-bounced stride-0 broadcast + one fused multiply normalize the
  gathered ctx.
* Attention-times-V keeps V plus a trailing ones column as the 65-column
  stationary operand and streams the exp tile; scores use tile_position
  row pairs so the two heads' score matmuls run concurrently; exp is one
  ACT instruction per key block covering both heads; the diagonal tri-mask
  is one DVE multiply per block via a stride-0 broadcast AP over heads.
* A tiny warm-up AllToAll is issued during the load phase so the first real
  collective doesn't pay the ~23 us first-call setup on the critical path.
"""

import sys

sys.path.insert(0, "/opt/trn_rl_repo")

import ml_dtypes
import numpy as np

import concourse.bass as bass
import concourse.tile as tile
from concourse import bacc, mybir
from concourse.bass_utils import run_bass_kernel_spmd

N_CORES = 8
B, S, H, D = 2, 2048, 16, 64
E = H * D            # 1024
T = B * S            # 4096 tokens
DPC = 128            # dims (2 heads) per core
NKC = E // 128       # 8 contraction chunks for the projections
SB = S // 128        # 16 key blocks per batch
PHQ = 512 // N_CORES  # 64 tokens per core per query group
PH = 2 * PHQ         # 128 tokens per core per half-batch
CR = 130             # a2a chunk rows: 2 x (64 ctx dims + den)

F32 = mybir.dt.float32
BF16 = mybir.dt.bfloat16
AFT = mybir.ActivationFunctionType

QT_ORDER = (0, 1, 3, 2)  # hf0 = {0,1} finishes early; q2 (12 blocks) last


def build_program():
    nc = bacc.Bacc("TRN2", target_bir_lowering=False, debug=False,
                   num_devices=N_CORES)

    xT = nc.dram_tensor("xT", [E, T], BF16, kind="ExternalInput").ap()
    wqT = nc.dram_tensor("wqT", [E, DPC], BF16, kind="ExternalInput").ap()
    wkT = nc.dram_tensor("wkT", [E, DPC], BF16, kind="ExternalInput").ap()
    wvT = nc.dram_tensor("wvT", [E, DPC], BF16, kind="ExternalInput").ap()
    woT = nc.dram_tensor("woT", [E, E], BF16, kind="ExternalInput").ap()
    bq = nc.dram_tensor("bq", [DPC, 1], F32, kind="ExternalInput").ap()
    bk = nc.dram_tensor("bk", [DPC, 1], F32, kind="ExternalInput").ap()
    bv = nc.dram_tensor("bv", [DPC, 1], F32, kind="ExternalInput").ap()
    bo = nc.dram_tensor("bo", [E], F32, kind="ExternalInput").ap()
    # single 128x128 lower-triangular (k_local <= q_local) mask
    tri = nc.dram_tensor("tri", [128, 128], BF16, kind="ExternalInput").ap()
    ident = nc.dram_tensor("ident", [128, 128], BF16, kind="ExternalInput").ap()
    out = nc.dram_tensor("out", [T // N_CORES, E], F32, kind="ExternalOutput").ap()

    with tile.TileContext(nc) as tc:
        with (
            tc.tile_pool(name="consts", bufs=1) as consts,
            tc.tile_pool(name="state", bufs=1) as state,
            tc.tile_pool(name="ep", bufs=6) as ep,
            tc.tile_pool(name="op", bufs=4) as op,
            tc.tile_pool(name="ps_p", bufs=2, space="PSUM") as ps_p,
            tc.tile_pool(name="ps_s", bufs=2, space="PSUM") as ps_s,
            tc.tile_pool(name="ps_c", bufs=2, space="PSUM") as ps_c,
            tc.tile_pool(name="dram", bufs=1, space="DRAM") as dram,
        ):
            # ---- warm-up collective: absorbs the first-AllToAll setup cost
            # while the DMA engines are still loading x ----------------------
            wu_s = consts.tile([128, 16], BF16)
            nc.vector.memset(wu_s[:], 0.0)
            wu_in = dram.tile([N_CORES, 16, 16], BF16, tag="wu_in", name="wu_in")
            wu_out = dram.tile([N_CORES, 16, 16], BF16, tag="wu_out",
                               name="wu_out")
            nc.sync.dma_start(out=wu_in[:], in_=wu_s[:])
            nc.gpsimd.collective_compute(
                "AllToAll",
                mybir.AluOpType.bypass,
                replica_groups=[list(range(N_CORES))],
                ins=[wu_in.opt()],
                outs=[wu_out.opt()],
            )

            def chunked(dram_ap, cols, kc0, kcn):
                # DRAM [E, cols] viewed as [p, kc, cols]: row kc*128+p
                return bass.AP(tensor=dram_ap.tensor,
                               offset=dram_ap.offset + kc0 * 128 * cols,
                               ap=[[cols, 128], [128 * cols, kcn], [1, cols]])

            # ---- loads in deadline order ---------------------------------
            wq_sb = consts.tile([128, NKC, DPC], BF16)
            bq_sb = consts.tile([128, 1], F32)
            nc.sync.dma_start(out=wq_sb[:], in_=chunked(wqT, DPC, 0, NKC))
            nc.sync.dma_start(out=bq_sb[:], in_=bq[:])
            # x tile 0 in two halves so the first matmul starts early
            x_t = [None] * NKC
            x0a = state.tile([128, NKC // 2, 512], BF16, name="x0a")
            x0b = state.tile([128, NKC // 2, 512], BF16, name="x0b")

            def x_ap(tt, kc0, kcn):
                return bass.AP(tensor=xT.tensor,
                               offset=xT.offset + tt * 512 + kc0 * 128 * T,
                               ap=[[T, 128], [128 * T, kcn], [1, 512]])

            nc.sync.dma_start(out=x0a[:], in_=x_ap(0, 0, 4))
            wk_sb = consts.tile([128, NKC, DPC], BF16)
            bk_sb = consts.tile([128, 1], F32)
            wv_sb = consts.tile([128, NKC, DPC], BF16)
            bv_sb = consts.tile([128, 1], F32)
            nc.sync.dma_start(out=wk_sb[:], in_=chunked(wkT, DPC, 0, NKC))
            nc.sync.dma_start(out=bk_sb[:], in_=bk[:])
            nc.sync.dma_start(out=x0b[:], in_=x_ap(0, 4, 4))
            nc.sync.dma_start(out=wv_sb[:], in_=chunked(wvT, DPC, 0, NKC))
            nc.sync.dma_start(out=bv_sb[:], in_=bv[:])
            tri_sb = consts.tile([128, 128], BF16)
            nc.sync.dma_start(out=tri_sb[:], in_=tri[:])
            id_sb = consts.tile([128, 128], BF16)
            nc.sync.dma_start(out=id_sb[:], in_=ident[:])
            for tt in range(1, NKC):
                xt = state.tile([128, NKC, 512], BF16, name=f"x{tt}")
                nc.sync.dma_start(out=xt[:], in_=x_ap(tt, 0, NKC))
                x_t[tt] = xt
            wo_sb = consts.tile([128, NKC, E], BF16)
            nc.sync.dma_start(out=wo_sb[:], in_=chunked(woT, E, 0, NKC))
            bo_bc = consts.tile([128, E], F32)
            nc.sync.dma_start(
                out=bo_bc[:],
                in_=bass.AP(tensor=bo.tensor, offset=bo.offset,
                            ap=[[0, 128], [1, E]]),
            )

            # ---- persistent activations -----------------------------------
            qT_sb = state.tile([128, T], BF16)   # [2-head dims, tokens]
            kT_sb = state.tile([128, T], BF16)
            vT_sb = state.tile([128, T], BF16)
            # per 128-token block: [64 v-dims, ones] per head -> the AV
            # matmul's 65-column stationary operand; the ones column makes
            # PSUM row 64 the softmax denominator.
            vN_sb = state.tile([128, T // 128, 130], BF16)
            # unnormalized ctx^T + den: rows 0-63 ctx dims, row 64 den
            ctx2_sb = state.tile([65, 2, T], BF16)

            nc.vector.memset(vN_sb[:, :, 64:65], 1.0)
            nc.vector.memset(vN_sb[:, :, 129:130], 1.0)

            # ---- stage builders -------------------------------------------
            def emit_proj(tt):
                ts = slice(tt * 512, (tt + 1) * 512)
                xa = (x0a, x0b) if tt == 0 else (x_t[tt],)
                nch = NKC // len(xa)

                ps_q = ps_p.tile([128, 512], F32, tag="p", name="ps_q")
                for kc in range(NKC):
                    xt = xa[kc // nch]
                    nc.tensor.matmul(ps_q[:], wq_sb[:, kc, :],
                                     xt[:, kc % nch, :],
                                     start=(kc == 0), stop=(kc == NKC - 1),
                                     skip_group_check=True)
                nc.vector.tensor_scalar_add(qT_sb[:, ts], ps_q[:], bq_sb[:])
                ps_k = ps_p.tile([128, 512], F32, tag="p", name="ps_k")
                for kc in range(NKC):
                    xt = xa[kc // nch]
                    nc.tensor.matmul(ps_k[:], wk_sb[:, kc, :],
                                     xt[:, kc % nch, :],
                                     start=(kc == 0), stop=(kc == NKC - 1),
                                     skip_group_check=True)
                nc.vector.tensor_scalar_add(kT_sb[:, ts], ps_k[:], bk_sb[:])
                ps_v = ps_p.tile([128, 512], F32, tag="p", name="ps_v")
                for kc in range(NKC):
                    xt = xa[kc // nch]
                    nc.tensor.matmul(ps_v[:], wv_sb[:, kc, :],
                                     xt[:, kc % nch, :],
                                     start=(kc == 0), stop=(kc == NKC - 1),
                                     skip_group_check=True)
                nc.vector.tensor_scalar_add(vT_sb[:, ts], ps_v[:], bv_sb[:])
                tp_ps = ps_p.tile([128, 4, 128], BF16, tag="p", name="tp_ps")
                for ti, tb in enumerate(range(tt * 4, tt * 4 + 4)):
                    nc.tensor.transpose(
                        tp_ps[:, ti, :], vT_sb[:, tb * 128:(tb + 1) * 128],
                        id_sb[:])
                    nc.vector.tensor_copy(vN_sb[:, tb, 0:64],
                                          tp_ps[:, ti, 0:64])
                    nc.vector.tensor_copy(vN_sb[:, tb, 65:129],
                                          tp_ps[:, ti, 64:128])

            def emit_attn(b, qt):
                t0 = b * S
                q0 = t0 + qt * 512
                nkb = 4 * qt + 4

                def emit_scores(kb):
                    c0 = max(kb - 4 * qt, 0) * 128
                    s = ps_s.tile([128, 2, 512], F32, tag="s", name="s_ps")
                    for h in range(2):
                        d0 = h * 64
                        nc.tensor.matmul(
                            s[:, h, c0:512],
                            kT_sb[d0:d0 + 64,
                                  t0 + kb * 128:t0 + (kb + 1) * 128],
                            qT_sb[d0:d0 + 64, q0 + c0:q0 + 512],
                            start=True, stop=True, skip_group_check=True)
                    return s

                s_tiles = {0: emit_scores(0)}
                cn = [ps_c.tile([128, 512], F32, tag="cn", name=f"cn{h}")
                      for h in range(2)]
                for kb in range(nkb):
                    m = kb - 4 * qt
                    c0 = max(m, 0) * 128
                    if kb + 1 < nkb:
                        s_tiles[kb + 1] = emit_scores(kb + 1)
                    s = s_tiles.pop(kb)
                    e = ep.tile([128, 2, 512], BF16, tag="e", name="e_sb")
                    nc.scalar.activation(e[:, :, c0:512], s[:, :, c0:512],
                                         AFT.Exp, scale=0.125)
                    if m >= 0:  # triangular block on the diagonal
                        nc.vector.tensor_mul(
                            e[:, :, c0:c0 + 128], e[:, :, c0:c0 + 128],
                            tri_sb[:].unsqueeze(1).broadcast_to((128, 2, 128)))
                    for h in range(2):
                        nc.tensor.matmul(
                            cn[h][0:65, c0:512],
                            vN_sb[:, b * SB + kb, 65 * h:65 * h + 65],
                            e[:, h, c0:512],
                            start=(kb == 0), stop=(kb == nkb - 1),
                            skip_group_check=True)

                # stage unnormalized ctx + den rows for the AllToAll
                for h in range(2):
                    nc.vector.tensor_copy(ctx2_sb[:, h, q0:q0 + 512],
                                          cn[h][0:65, :])

            def emit_half_a2a(b, hf):
                # half-batch = query groups {2hf, 2hf+1}; peer j gets this
                # core's heads' ctx for j's 128 half-batch tokens.
                base = b * S + hf * (S // 2)
                ctxd = dram.tile([N_CORES, CR, PH], BF16, tag="ctxd",
                                 name="ctxd", bufs=4)
                for h in range(2):
                    nc.sync.dma_start(
                        out=bass.AP(tensor=ctxd.tensor,
                                    offset=ctxd[0].offset + h * 65 * PH,
                                    ap=[[PH, 65], [CR * PH, N_CORES],
                                        [1, PH]]),
                        in_=ctx2_sb[:, h, base:base + S // 2].rearrange(
                            "p (j t) -> p j t", j=N_CORES))
                recv = dram.tile([N_CORES, CR, PH], BF16, tag="recv",
                                 name="recv", bufs=4)
                nc.gpsimd.collective_compute(
                    "AllToAll",
                    mybir.AluOpType.bypass,
                    replica_groups=[list(range(N_CORES))],
                    ins=[ctxd.opt()],
                    outs=[recv.opt()],
                )
                return recv

            def emit_half_recv(b, hf, recv):
                # gather + normalize the received ctx for one half-batch;
                # no PE work in this chain.
                cg_sb = op.tile([128, NKC, PH], BF16, tag="cg_sb", name="cg_sb",
                                bufs=2)
                den16 = op.tile([16, PH], BF16, tag="den16", name="den16",
                                bufs=2)
                r0 = recv[0]
                for h in range(2):
                    nc.sync.dma_start(
                        out=cg_sb[h * 64:(h + 1) * 64, :, :],
                        in_=bass.AP(tensor=r0.tensor,
                                    offset=r0.offset + h * 65 * PH,
                                    ap=[[PH, 64], [CR * PH, N_CORES],
                                        [1, PH]]))
                nc.sync.dma_start(
                    out=den16[:],
                    in_=bass.AP(tensor=r0.tensor,
                                offset=r0.offset + 64 * PH,
                                ap=[[CR * PH, N_CORES], [65 * PH, 2],
                                    [1, PH]]))
                r16 = op.tile([16, PH], F32, tag="r16", name="r16", bufs=2)
                nc.vector.reciprocal(r16[:], den16[:])
                rd = dram.tile([16, PH], F32, tag="rd", name="rd", bufs=4)
                nc.sync.dma_start(out=rd[:], in_=r16[:])
                rmap = op.tile([128, NKC, PH], F32, tag="rmap", name="rmap",
                               bufs=2)
                rd0 = rd[0]
                for h in range(2):
                    nc.sync.dma_start(
                        out=rmap[h * 64:(h + 1) * 64, :, :],
                        in_=bass.AP(tensor=rd0.tensor,
                                    offset=rd0.offset + h * PH,
                                    ap=[[0, 64], [2 * PH, N_CORES], [1, PH]]))
                nc.vector.tensor_mul(cg_sb[:], cg_sb[:], rmap[:])
                return b, hf, cg_sb

            def emit_half_wo(b, hf, cg_sb):
                o_sb = op.tile([PH, E], F32, tag="o_sb", name="o_sb")
                for et in range(2):
                    ps = ps_s.tile([128, 2, 512], F32, tag="s", name="c_ps")
                    for kc in range(NKC):
                        nc.tensor.matmul(
                            ps[0:PH, 0, :],
                            cg_sb[:, kc, :],
                            wo_sb[:, kc, et * 512:(et + 1) * 512],
                            start=(kc == 0), stop=(kc == NKC - 1),
                            skip_group_check=True)
                    nc.vector.tensor_add(
                        o_sb[:, et * 512:(et + 1) * 512], ps[0:PH, 0, :],
                        bo_bc[0:PH, et * 512:(et + 1) * 512])
                r0w = (b * 2 + hf) * PH
                nc.sync.dma_start(out=out[r0w:r0w + PH, :], in_=o_sb[:])

            # ---- schedule -------------------------------------------------
            # qt order {0,1,3,2} per batch: half 0's a2a fires after 12
            # key-blocks of work, half 1's after all 40.  4 collectives
            # total, spaced ~40 us apart (back-to-back a2as degrade ~3x on
            # this part, and the first collective completes no earlier than
            # ~100 us because of the first-call barrier) so every
            # collective-dependent Wo block is placed with >=20 us of slack
            # after its gating collective's expected completion; the last
            # two independent Wo blocks are held back to keep the PE busy
            # through the final collective.
            emit_proj(0)
            # -------- batch 0
            emit_attn(0, 0)
            emit_proj(1)
            emit_attn(0, 1)
            rA = emit_half_a2a(0, 0)          # cc1, trigger ~50us
            emit_proj(2)
            emit_proj(3)
            emit_attn(0, 3)
            emit_proj(4)
            emit_attn(0, 2)
            rB = emit_half_a2a(0, 1)          # cc2, trigger ~85us
            emit_proj(5)
            # -------- batch 1
            emit_attn(1, 0)
            emit_proj(6)
            args00 = emit_half_recv(0, 0, rA)  # cc1 done ~100us
            emit_attn(1, 1)
            rC = emit_half_a2a(1, 0)          # cc3, trigger ~130us
            emit_proj(7)
            emit_half_wo(*args00)             # PE reaches here ~125us
            args01 = emit_half_recv(0, 1, rB)  # cc2 done ~110us
            emit_attn(1, 3)
            emit_attn(1, 2)
            rD = emit_half_a2a(1, 1)          # cc4 (tail), trigger ~165us
            # tail: two independent Wo blocks overlap the final collective
            emit_half_wo(*args01)
            args10 = emit_half_recv(1, 0, rC)  # cc3 done ~150us
            emit_half_wo(*args10)
            args11 = emit_half_recv(1, 1, rD)
            emit_half_wo(*args11)

    nc.compile()
    return nc


_NC = None


def _get_program():
    global _NC
    if _NC is None:
        _NC = build_program()
    return _NC


def _bf(a):
    return np.ascontiguousarray(a).astype(ml_dtypes.bfloat16)


def kernel(x, Wq, bq, Wk, bk, Wv, bv, Wo, bo, _trace=False, _trace_kwargs=None):
    x = np.asarray(x, np.float32)
    Wq, Wk, Wv, Wo = (np.asarray(w, np.float32) for w in (Wq, Wk, Wv, Wo))
    bq, bk, bv, bo = (np.asarray(v, np.float32) for v in (bq, bk, bv, bo))

    xT = _bf(x.reshape(T, E).T)
    i = np.arange(128)
    tri = _bf((i[:, None] <= i[None, :]).astype(np.float32))
    ident = _bf(np.eye(128, dtype=np.float32))

    in_maps = []
    for c in range(N_CORES):
        sl = slice(c * DPC, (c + 1) * DPC)
        in_maps.append({
            "xT": xT,
            "wqT": _bf(Wq[sl, :].T),
            "wkT": _bf(Wk[sl, :].T),
            "wvT": _bf(Wv[sl, :].T),
            "woT": _bf(Wo.T),
            "bq": bq[sl].reshape(DPC, 1).copy(),
            "bk": bk[sl].reshape(DPC, 1).copy(),
            "bv": bv[sl].reshape(DPC, 1).copy(),
            "bo": bo,
            "tri": tri,
            "ident": ident,
        })

    nc = _get_program()
    res = run_bass_kernel_spmd(nc, in_maps, list(range(N_CORES)),
                               trace=_trace, **(_trace_kwargs or {}))
    # out[c] rows are [batch, half, 128]: row (b, hf, r) holds global
    # token b*2048 + hf*1024 + c*128 + r.
    stacked = np.stack([res.results[i]["out"].reshape(B, 2, PH, E)
                        for i in range(N_CORES)], axis=2)
    full = stacked.reshape(T, E)
    if _trace:
        return full.reshape(B, S, E), res
    return full.reshape(B, S, E)


# revision 10
# speedup vs baseline: 1.0714x; 1.0109x over previous
"""Multi-head attention (B=2, S=2048, H=16, D=64) on 8 Trainium2 NeuronCores.

Head-parallel tensor parallelism: core c owns heads {2c, 2c+1} (a 128-dim
slice of the model dim): column-parallel QKV projections and local causal
attention for its 2 heads, then an AllToAll of bf16 context vectors (one
512-token query group at a time) and a full-width Wo projection for this
core's own disjoint 64-token output slices.

Schedule (v2), shaped by trace measurements:

* Startup: wq/bq and the first half of x's token-tile 0 are the first DMAs
  issued (the x0 tile is split in two so the first projection matmuls start
  after ~0.75 MB of transfers instead of ~3 MB).  Everything else loads
  behind them in deadline order (tri before the first diagonal block,
  wo/bo last).
* Query groups run in order {0, 1, 3, 2} per batch; each half-batch's
  unnormalized ctx + softmax denominators are AllToAll'd right when its
  second query group finishes.  4 collectives, spaced ~40 us: per-qt
  granularity (8 collectives) was measured to DEGRADE - back-to-back
  AllToAlls on this part grow from ~6 us to ~22 us each, and the first
  collective completes no earlier than ~100 us (first-call barrier), so
  every collective-dependent Wo block is placed with >=20 us of slack and
  the last two independent Wo blocks are held back as PE filler for the
  final collective.
* PSUM: proj pool 2 banks (q/k/v/transpose rotate through [128,512] slots),
  scores 2x2 banks, ctx accumulators 2 banks = exactly 8.  The projection
  for tile t+1 and the next half-batch's Wo matmuls are emitted after the
  attention section that hides them; the Tile scheduler slots them into
  the PE stalls where attention waits on the ACT exp stream (exp on ACT is
  ~1.15 us per 128-key block vs ~0.65 us of PE work, so without filler the
  PE idles ~40% during attention and the HAM clock gate re-throttles).
* batch-1 half-0's Wo matmuls are emitted after the LAST a2a is issued so
  the PE stays busy (and warm) through the final collective.
* Softmax normalization happens on the receiving core (the a2a payload is
  65 rows per head: 64 unnormalized ctx dims + the denominator row from a
  trailing ones-column in the AV stationary); 16 denominator rows stack on
  the partition axis at the receiver where one 16-lane DVE reciprocal + a
  DRAM-bounced stride-0 broadcast + one fused multiply normalize the
  gathered ctx.
* Attention-times-V keeps V plus a trailing ones column as the 65-column
  stationary operand and streams the exp tile; scores use tile_position
  row pairs so the two heads' score matmuls run concurrently; exp is one
  ACT instruction per key block covering both heads; the diagonal tri-mask
  is one DVE multiply per block via a stride-0 broadcast AP over heads.
* A tiny warm-up AllToAll is issued during the load phase so the first real
  collective doesn't pay the ~23 us first-call setup on the critical path.
"""

import sys

sys.path.insert(0, "/opt/trn_rl_repo")

import ml_dtypes
import numpy as np

import concourse.bass as bass
import concourse.tile as tile
from concourse import bacc, mybir
from concourse.bass_utils import run_bass_kernel_spmd

N_CORES = 8
B, S, H, D = 2, 2048, 16, 64
E = H * D            # 1024
T = B * S            # 4096 tokens
DPC = 128            # dims (2 heads) per core
NKC = E // 128       # 8 contraction chunks for the projections
SB = S // 128        # 16 key blocks per batch
PHQ = 512 // N_CORES  # 64 tokens per core per query group
PH = 2 * PHQ         # 128 tokens per core per half-batch
CR = 130             # a2a chunk rows: 2 x (64 ctx dims + den)

F32 = mybir.dt.float32
BF16 = mybir.dt.bfloat16
AFT = mybir.ActivationFunctionType

QT_ORDER = (0, 1, 3, 2)  # hf0 = {0,1} finishes early; q2 (12 blocks) last


def build_program():
    nc = bacc.Bacc("TRN2", target_bir_lowering=False, debug=False,
                   num_devices=N_CORES)

    xT = nc.dram_tensor("xT", [E, T], BF16, kind="ExternalInput").ap()
    wqT = nc.dram_tensor("wqT", [E, DPC], BF16, kind="ExternalInput").ap()
    wkT = nc.dram_tensor("wkT", [E, DPC], BF16, kind="ExternalInput").ap()
    wvT = nc.dram_tensor("wvT", [E, DPC], BF16, kind="ExternalInput").ap()
    woT = nc.dram_tensor("woT", [E, E], BF16, kind="ExternalInput").ap()
    bq = nc.dram_tensor("bq", [DPC, 1], F32, kind="ExternalInput").ap()
    bk = nc.dram_tensor("bk", [DPC, 1], F32, kind="ExternalInput").ap()
    bv = nc.dram_tensor("bv", [DPC, 1], F32, kind="ExternalInput").ap()
    bo = nc.dram_tensor("bo", [E], F32, kind="ExternalInput").ap()
    # single 128x128 lower-triangular (k_local <= q_local) mask
    tri = nc.dram_tensor("tri", [128, 128], BF16, kind="ExternalInput").ap()
    ident = nc.dram_tensor("ident", [128, 128], BF16, kind="ExternalInput").ap()
    out = nc.dram_tensor("out", [T // N_CORES, E], F32, kind="ExternalOutput").ap()

    with tile.TileContext(nc) as tc:
        with (
            tc.tile_pool(name="consts", bufs=1) as consts,
            tc.tile_pool(name="state", bufs=1) as state,
            tc.tile_pool(name="ep", bufs=6) as ep,
            tc.tile_pool(name="op", bufs=4) as op,
            tc.tile_pool(name="ps_p", bufs=2, space="PSUM") as ps_p,
            tc.tile_pool(name="ps_s", bufs=2, space="PSUM") as ps_s,
            tc.tile_pool(name="ps_c", bufs=2, space="PSUM") as ps_c,
            tc.tile_pool(name="dram", bufs=1, space="DRAM") as dram,
        ):
            # ---- warm-up collective: absorbs the first-AllToAll setup cost
            # while the DMA engines are still loading x ----------------------
            wu_s = consts.tile([128, 16], BF16)
            nc.vector.memset(wu_s[:], 0.0)
            wu_in = dram.tile([N_CORES, 16, 16], BF16, tag="wu_in", name="wu_in")
            wu_out = dram.tile([N_CORES, 16, 16], BF16, tag="wu_out",
                               name="wu_out")
            nc.sync.dma_start(out=wu_in[:], in_=wu_s[:])
            nc.gpsimd.collective_compute(
                "AllToAll",
                mybir.AluOpType.bypass,
                replica_groups=[list(range(N_CORES))],
                ins=[wu_in.opt()],
                outs=[wu_out.opt()],
            )

            def chunked(dram_ap, cols, kc0, kcn):
                # DRAM [E, cols] viewed as [p, kc, cols]: row kc*128+p
                return bass.AP(tensor=dram_ap.tensor,
                               offset=dram_ap.offset + kc0 * 128 * cols,
                               ap=[[cols, 128], [128 * cols, kcn], [1, cols]])

            # ---- loads in deadline order ---------------------------------
            wq_sb = consts.tile([128, NKC, DPC], BF16)
            bq_sb = consts.tile([128, 1], F32)
            nc.sync.dma_start(out=wq_sb[:], in_=chunked(wqT, DPC, 0, NKC))
            nc.sync.dma_start(out=bq_sb[:], in_=bq[:])
            # x tile 0 in two halves so the first matmul starts early
            x_t = [None] * NKC
            x0a = state.tile([128, NKC // 2, 512], BF16, name="x0a")
            x0b = state.tile([128, NKC // 2, 512], BF16, name="x0b")

            def x_ap(tt, kc0, kcn):
                return bass.AP(tensor=xT.tensor,
                               offset=xT.offset + tt * 512 + kc0 * 128 * T,
                               ap=[[T, 128], [128 * T, kcn], [1, 512]])

            nc.sync.dma_start(out=x0a[:], in_=x_ap(0, 0, 4))
            wk_sb = consts.tile([128, NKC, DPC], BF16)
            bk_sb = consts.tile([128, 1], F32)
            wv_sb = consts.tile([128, NKC, DPC], BF16)
            bv_sb = consts.tile([128, 1], F32)
            nc.sync.dma_start(out=wk_sb[:], in_=chunked(wkT, DPC, 0, NKC))
            nc.sync.dma_start(out=bk_sb[:], in_=bk[:])
            nc.sync.dma_start(out=x0b[:], in_=x_ap(0, 4, 4))
            nc.sync.dma_start(out=wv_sb[:], in_=chunked(wvT, DPC, 0, NKC))
            nc.sync.dma_start(out=bv_sb[:], in_=bv[:])
            tri_sb = consts.tile([128, 128], BF16)
            nc.sync.dma_start(out=tri_sb[:], in_=tri[:])
            id_sb = consts.tile([128, 128], BF16)
            nc.sync.dma_start(out=id_sb[:], in_=ident[:])
            for tt in range(1, NKC):
                xt = state.tile([128, NKC, 512], BF16, name=f"x{tt}")
                nc.sync.dma_start(out=xt[:], in_=x_ap(tt, 0, NKC))
                x_t[tt] = xt
            wo_sb = consts.tile([128, NKC, E], BF16)
            nc.sync.dma_start(out=wo_sb[:], in_=chunked(woT, E, 0, NKC))
            bo_bc = consts.tile([128, E], F32)
            nc.sync.dma_start(
                out=bo_bc[:],
                in_=bass.AP(tensor=bo.tensor, offset=bo.offset,
                            ap=[[0, 128], [1, E]]),
            )

            # ---- persistent activations -----------------------------------
            qT_sb = state.tile([128, T], BF16)   # [2-head dims, tokens]
            kT_sb = state.tile([128, T], BF16)
            vT_sb = state.tile([128, T], BF16)
            # per 128-token block: [64 v-dims, ones] per head -> the AV
            # matmul's 65-column stationary operand; the ones column makes
            # PSUM row 64 the softmax denominator.
            vN_sb = state.tile([128, T // 128, 130], BF16)
            # unnormalized ctx^T + den: rows 0-63 ctx dims, row 64 den
            ctx2_sb = state.tile([65, 2, T], BF16)

            nc.vector.memset(vN_sb[:, :, 64:65], 1.0)
            nc.vector.memset(vN_sb[:, :, 129:130], 1.0)

            # ---- stage builders -------------------------------------------
            def emit_proj(tt):
                ts = slice(tt * 512, (tt + 1) * 512)
                xa = (x0a, x0b) if tt == 0 else (x_t[tt],)
                nch = NKC // len(xa)

                ps_q = ps_p.tile([128, 512], F32, tag="p", name="ps_q")
                for kc in range(NKC):
                    xt = xa[kc // nch]
                    nc.tensor.matmul(ps_q[:], wq_sb[:, kc, :],
                                     xt[:, kc % nch, :],
                                     start=(kc == 0), stop=(kc == NKC - 1),
                                     skip_group_check=True)
                nc.vector.tensor_scalar_add(qT_sb[:, ts], ps_q[:], bq_sb[:])
                ps_k = ps_p.tile([128, 512], F32, tag="p", name="ps_k")
                for kc in range(NKC):
                    xt = xa[kc // nch]
                    nc.tensor.matmul(ps_k[:], wk_sb[:, kc, :],
                                     xt[:, kc % nch, :],
                                     start=(kc == 0), stop=(kc == NKC - 1),
                                     skip_group_check=True)
                nc.vector.tensor_scalar_add(kT_sb[:, ts], ps_k[:], bk_sb[:])
                ps_v = ps_p.tile([128, 512], F32, tag="p", name="ps_v")
                for kc in range(NKC):
                    xt = xa[kc // nch]
                    nc.tensor.matmul(ps_v[:], wv_sb[:, kc, :],
                                     xt[:, kc % nch, :],
                                     start=(kc == 0), stop=(kc == NKC - 1),
                                     skip_group_check=True)
                nc.vector.tensor_scalar_add(vT_sb[:, ts], ps_v[:], bv_sb[:])
                tp_ps = ps_p.tile([128, 4, 128], BF16, tag="p", name="tp_ps")
                for ti, tb in enumerate(range(tt * 4, tt * 4 + 4)):
                    nc.tensor.transpose(
                        tp_ps[:, ti, :], vT_sb[:, tb * 128:(tb + 1) * 128],
                        id_sb[:])
                    nc.vector.tensor_copy(vN_sb[:, tb, 0:64],
                                          tp_ps[:, ti, 0:64])
                    nc.vector.tensor_copy(vN_sb[:, tb, 65:129],
                                          tp_ps[:, ti, 64:128])

            def emit_attn(b, qt):
                t0 = b * S
                q0 = t0 + qt * 512
                nkb = 4 * qt + 4

                def emit_scores(kb):
                    c0 = max(kb - 4 * qt, 0) * 128
                    s = ps_s.tile([128, 2, 512], F32, tag="s", name="s_ps")
                    for h in range(2):
                        d0 = h * 64
                        nc.tensor.matmul(
                            s[:, h, c0:512],
                            kT_sb[d0:d0 + 64,
                                  t0 + kb * 128:t0 + (kb + 1) * 128],
                            qT_sb[d0:d0 + 64, q0 + c0:q0 + 512],
                            start=True, stop=True, skip_group_check=True)
                    return s

                s_tiles = {0: emit_scores(0)}
                cn = [ps_c.tile([128, 512], F32, tag="cn", name=f"cn{h}")
                      for h in range(2)]
                for kb in range(nkb):
                    m = kb - 4 * qt
                    c0 = max(m, 0) * 128
                    if kb + 1 < nkb:
                        s_tiles[kb + 1] = emit_scores(kb + 1)
                    s = s_tiles.pop(kb)
                    e = ep.tile([128, 2, 512], BF16, tag="e", name="e_sb")
                    nc.scalar.activation(e[:, :, c0:512], s[:, :, c0:512],
                                         AFT.Exp, scale=0.125)
                    if m >= 0:  # triangular block on the diagonal
                        nc.vector.tensor_mul(
                            e[:, :, c0:c0 + 128], e[:, :, c0:c0 + 128],
                            tri_sb[:].unsqueeze(1).broadcast_to((128, 2, 128)))
                    for h in range(2):
                        nc.tensor.matmul(
                            cn[h][0:65, c0:512],
                            vN_sb[:, b * SB + kb, 65 * h:65 * h + 65],
                            e[:, h, c0:512],
                            start=(kb == 0), stop=(kb == nkb - 1),
                            skip_group_check=True)

                # stage unnormalized ctx + den rows for the AllToAll
                for h in range(2):
                    nc.vector.tensor_copy(ctx2_sb[:, h, q0:q0 + 512],
                                          cn[h][0:65, :])

            def emit_half_a2a(b, hf):
                # half-batch = query groups {2hf, 2hf+1}; peer j gets this
                # core's heads' ctx for j's 128 half-batch tokens.
                base = b * S + hf * (S // 2)
                ctxd = dram.tile([N_CORES, CR, PH], BF16, tag="ctxd",
                                 name="ctxd", bufs=4)
                for h in range(2):
                    nc.sync.dma_start(
                        out=bass.AP(tensor=ctxd.tensor,
                                    offset=ctxd[0].offset + h * 65 * PH,
                                    ap=[[PH, 65], [CR * PH, N_CORES],
                                        [1, PH]]),
                        in_=ctx2_sb[:, h, base:base + S // 2].rearrange(
                            "p (j t) -> p j t", j=N_CORES))
                recv = dram.tile([N_CORES, CR, PH], BF16, tag="recv",
                                 name="recv", bufs=4)
                nc.gpsimd.collective_compute(
                    "AllToAll",
                    mybir.AluOpType.bypass,
                    replica_groups=[list(range(N_CORES))],
                    ins=[ctxd.opt()],
                    outs=[recv.opt()],
                )
                return recv

            def emit_half_recv(b, hf, recv):
                # gather + normalize the received ctx for one half-batch;
                # no PE work in this chain.
                cg_sb = op.tile([128, NKC, PH], BF16, tag="cg_sb", name="cg_sb",
                                bufs=2)
                den16 = op.tile([16, PH], BF16, tag="den16", name="den16",
                                bufs=2)
                r0 = recv[0]
                for h in range(2):
                    nc.sync.dma_start(
                        out=cg_sb[h * 64:(h + 1) * 64, :, :],
                        in_=bass.AP(tensor=r0.tensor,
                                    offset=r0.offset + h * 65 * PH,
                                    ap=[[PH, 64], [CR * PH, N_CORES],
                                        [1, PH]]))
                nc.sync.dma_start(
                    out=den16[:],
                    in_=bass.AP(tensor=r0.tensor,
                                offset=r0.offset + 64 * PH,
                                ap=[[CR * PH, N_CORES], [65 * PH, 2],
                                    [1, PH]]))
                r16 = op.tile([16, PH], F32, tag="r16", name="r16", bufs=2)
                nc.vector.reciprocal(r16[:], den16[:])
                rd = dram.tile([16, PH], F32, tag="rd", name="rd", bufs=4)
                nc.sync.dma_start(out=rd[:], in_=r16[:])
                rmap = op.tile([128, NKC, PH], F32, tag="rmap", name="rmap",
                               bufs=2)
                rd0 = rd[0]
                for h in range(2):
                    nc.sync.dma_start(
                        out=rmap[h * 64:(h + 1) * 64, :, :],
                        in_=bass.AP(tensor=rd0.tensor,
                                    offset=rd0.offset + h * PH,
                                    ap=[[0, 64], [2 * PH, N_CORES], [1, PH]]))
                nc.vector.tensor_mul(cg_sb[:], cg_sb[:], rmap[:])
                return b, hf, cg_sb

            def emit_half_wo(b, hf, cg_sb):
                o_sb = op.tile([PH, E], F32, tag="o_sb", name="o_sb")
                for et in range(2):
                    ps = ps_s.tile([128, 2, 512], F32, tag="s", name="c_ps")
                    for kc in range(NKC):
                        nc.tensor.matmul(
                            ps[0:PH, 0, :],
                            cg_sb[:, kc, :],
                            wo_sb[:, kc, et * 512:(et + 1) * 512],
                            start=(kc == 0), stop=(kc == NKC - 1),
                            skip_group_check=True)
                    nc.vector.tensor_add(
                        o_sb[:, et * 512:(et + 1) * 512], ps[0:PH, 0, :],
                        bo_bc[0:PH, et * 512:(et + 1) * 512])
                r0w = (b * 2 + hf) * PH
                nc.sync.dma_start(out=out[r0w:r0w + PH, :], in_=o_sb[:])

            # ---- schedule -------------------------------------------------
            # qt order {2,3,0,1} per batch: half 1 (28 key-blocks of work)
            # finishes first and its a2a fires ~70 us before the end; half 0
            # ({q0,q1}, only 12 key-blocks) finishes last so the FINAL
            # collective fires ~15 us of attention + ~15 us of held-back Wo
            # work before the PE drains.  4 collectives, spaced >=15 us
            # (back-to-back a2as degrade ~3x on this part) and the first
            # completes no earlier than ~100 us (first-call barrier), so
            # every collective-dependent Wo block is placed with >=20 us of
            # slack after its gating collective's expected completion.
            emit_proj(0)
            emit_proj(1)
            emit_proj(2)
            # -------- batch 0
            emit_attn(0, 2)
            emit_proj(3)
            emit_attn(0, 3)
            rB = emit_half_a2a(0, 1)          # cc1, trigger ~55us
            emit_attn(0, 0)
            emit_proj(4)
            emit_attn(0, 1)
            rA = emit_half_a2a(0, 0)          # cc2, trigger ~75us
            emit_proj(5)
            emit_proj(6)
            # -------- batch 1
            emit_attn(1, 2)
            emit_proj(7)
            args01 = emit_half_recv(0, 1, rB)  # cc1 done ~105us
            emit_attn(1, 3)
            rD = emit_half_a2a(1, 1)          # cc3, trigger ~125us
            emit_half_wo(*args01)             # PE ~130us
            emit_attn(1, 0)
            args00 = emit_half_recv(0, 0, rA)  # cc2 done ~110us
            emit_attn(1, 1)
            rC = emit_half_a2a(1, 0)          # cc4 (tail), trigger ~142us
            # tail: independent Wo blocks keep the PE busy through the
            # final collective + its recv chain
            emit_half_wo(*args00)
            args11 = emit_half_recv(1, 1, rD)  # cc3 done ~137us
            emit_half_wo(*args11)
            args10 = emit_half_recv(1, 0, rC)
            emit_half_wo(*args10)

    nc.compile()
    return nc


_NC = None


def _get_program():
    global _NC
    if _NC is None:
        _NC = build_program()
    return _NC


def _bf(a):
    return np.ascontiguousarray(a).astype(ml_dtypes.bfloat16)


def kernel(x, Wq, bq, Wk, bk, Wv, bv, Wo, bo, _trace=False, _trace_kwargs=None):
    x = np.asarray(x, np.float32)
    Wq, Wk, Wv, Wo = (np.asarray(w, np.float32) for w in (Wq, Wk, Wv, Wo))
    bq, bk, bv, bo = (np.asarray(v, np.float32) for v in (bq, bk, bv, bo))

    xT = _bf(x.reshape(T, E).T)
    i = np.arange(128)
    tri = _bf((i[:, None] <= i[None, :]).astype(np.float32))
    ident = _bf(np.eye(128, dtype=np.float32))

    in_maps = []
    for c in range(N_CORES):
        sl = slice(c * DPC, (c + 1) * DPC)
        in_maps.append({
            "xT": xT,
            "wqT": _bf(Wq[sl, :].T),
            "wkT": _bf(Wk[sl, :].T),
            "wvT": _bf(Wv[sl, :].T),
            "woT": _bf(Wo.T),
            "bq": bq[sl].reshape(DPC, 1).copy(),
            "bk": bk[sl].reshape(DPC, 1).copy(),
            "bv": bv[sl].reshape(DPC, 1).copy(),
            "bo": bo,
            "tri": tri,
            "ident": ident,
        })

    nc = _get_program()
    res = run_bass_kernel_spmd(nc, in_maps, list(range(N_CORES)),
                               trace=_trace, **(_trace_kwargs or {}))
    # out[c] rows are [batch, half, 128]: row (b, hf, r) holds global
    # token b*2048 + hf*1024 + c*128 + r.
    stacked = np.stack([res.results[i]["out"].reshape(B, 2, PH, E)
                        for i in range(N_CORES)], axis=2)
    full = stacked.reshape(T, E)
    if _trace:
        return full.reshape(B, S, E), res
    return full.reshape(B, S, E)
